# revision 13
# baseline (speedup 1.0000x reference)
"""MemoryBank.get_all_distances Trainium2 kernel.

emb_batch [64, 64] f32, bank [131072, 64] f32 -> distances [64, 131072] f32
  distances[n, b] = || bank[b] - emb[n] ||_2

Strategy: shard bank rows across 8 cores (16384 rows each). On the host we
only re-layout (transpose + stack + bf16 cast) the shard; all arithmetic
runs on device:

  dist^2[n, b] = (||e_n||^2 + ||b_b||^2) - 2 e_n . b_b

MemoryBank rows are L2-normalized (see MemoryBank._create), so ||b_b||^2
== 1 and the first term is the per-query constant ||e_n||^2 + 1, computed
on device (DVE square + reduce + add). kernel() verifies the normalization
on the host (read-only check) and falls back to a variant that computes
||b_b||^2 on device (DVE squares + ones-matmul) if it does not hold.

Per core the shard is fed as bt [128, 8192] bf16: partitions 0-63 hold dim
d of bank columns 0..8191 of the shard, partitions 64-127 of columns
8192..16383. Each 512-column block is one K=128/M=128 bf16 matmul with the
block-diagonal stationary [[-2*embT,0],[0,-2*embT]]; the ACT engine
finishes with sqrt(psum + bias) -> bf16 and the host widens to f32.

Scheduling notes (measured from perfetto traces):
 - the NRT preamble runs to ~6.5us and a DMA doorbell costs ~0.65us of the
   issuing engine, with ~1.5us to first byte and ~2.1us from last byte to
   the completion semaphore. Both HWDGE rings sustain ~240 GB/s.
 - the ACT-table loads occupy the scalar ring until ~8.5us, so all early
   traffic rides the sync ring; the scalar ring gets late chunks only.
 - embeddings travel as ONE [128, 192] tensor (tiled embT + query-major
   copy): separate small DMAs with <512B lines crawl and serialize the
   ring.
 - chunk widths are graded (512 first / last) so the serial ACT-sqrt chain
   starts as early and drains as late as possible; chunks are processed in
   measured data-availability order, not offset order.
 - output tiles use bufs=6: an output DMA's HBM write receipt (~2us) holds
   its tile hostage well past the transfer.
 - the PE boots HAM-throttled at half clock; 3 dummy matmuls warm it up.
"""

import numpy as np

BANK = 131072
DIM = 64
BATCH = 64
N_CORES = 8
SHARD = BANK // N_CORES  # 16384 bank rows per core
HALF = SHARD // 2  # 8192 columns per partition-half
NBLK = 512  # psum bank / matmul block
N_WARM = 3  # PE warm-up dummy matmuls

# chunk widths (columns per partition-half); sum == HALF
CHUNKS = [512, 512, 1024, 1024, 1024, 1024, 1024, 1024, 512, 512]
# DMA groups as (engine, [chunk indices]): each DMA costs ~1.7us of its
# ring serially plus ~1.7us to its completion semaphore, so the fixed
# costs are spread across all three queues and run in parallel.
GROUPS = [
    ("sync", [0]),
    ("scalar", [1, 2, 3]),
    ("scalar", [4, 5]),
    ("gpsimd", [6, 7, 8, 9]),
]
# processing order (availability is balanced for natural order)
ORDER = [0, 1, 2, 3, 4, 5, 6, 7, 8, 9]

_cache = {}

# test.py reads this after calling kernel() to get profiling info.
last_run = None


def _build(fused_norm=True, half=HALF):
    import concourse.mybir as mybir
    import concourse.tile as tile
    from concourse import bacc

    f32 = mybir.dt.float32
    bf16 = mybir.dt.bfloat16
    SQRT = mybir.ActivationFunctionType.Sqrt

    offs = np.concatenate([[0], np.cumsum(CHUNKS)])
    assert offs[-1] == half

    nc = bacc.Bacc(
        "TRN2", target_bir_lowering=False, debug=False, num_devices=N_CORES
    )
    bt = nc.dram_tensor("bt", [128, half], bf16, kind="ExternalInput").ap()
    # ewc: cols 0-127 = tiled embT (stationary), 128-191 = query-major emb.
    ewc = nc.dram_tensor("ewc", [128, 192], f32, kind="ExternalInput").ap()
    o = nc.dram_tensor("o", [128, half], bf16, kind="ExternalOutput").ap()

    with tile.TileContext(nc) as tc:
        with (
            tc.tile_pool(name="singles", bufs=1) as singles,
            tc.tile_pool(name="bt_pool", bufs=1) as bt_pool,
            tc.tile_pool(name="sq_pool", bufs=3) as sq_pool,
            tc.tile_pool(name="out_pool", bufs=6) as out_pool,
            tc.tile_pool(name="psum", bufs=4, space="PSUM") as psum,
        ):
            # --- input streams, all doorbells rung up front ---------------
            ewc2 = singles.tile([128, 192], f32)
            nc.gpsimd.dma_start(out=ewc2, in_=ewc)

            # Preload the Sqrt ACT table with the production signature.
            zt = singles.tile([128, 1], f32)
            nc.vector.memset(zt, 0.0)
            warm = singles.tile([128, 1], f32)
            nc.scalar.activation(out=warm, in_=zt, func=SQRT, bias=zt, scale=1.0)

            bts = {}
            for gi, (eng_name, cis) in enumerate(GROUPS):
                lo, hi = int(offs[cis[0]]), int(offs[cis[-1] + 1])
                gt = bt_pool.tile(
                    [128, hi - lo], bf16, name=f"btg{gi}", tag=f"btg{gi}"
                )
                for ci in cis:
                    bts[ci] = gt[:, int(offs[ci]) - lo : int(offs[ci + 1]) - lo]
                eng = {"sync": nc.sync, "scalar": nc.scalar,
                       "gpsimd": nc.gpsimd}[eng_name]
                eng.dma_start(out=gt, in_=bt[:, lo:hi])

            # --- PE warm-up against the HAM clock gate --------------------
            dummy_w = singles.tile([128, 128], bf16)
            nc.gpsimd.memset(dummy_w, 0.0)
            dummy_r = singles.tile([128, 512], bf16)
            nc.gpsimd.memset(dummy_r, 0.0)
            ps_warm = psum.tile([128, 1024], f32, tag="ps")
            for _ in range(N_WARM):
                nc.tensor.matmul(
                    ps_warm[:, 0:512], lhsT=dummy_w, rhs=dummy_r,
                    start=True, stop=True,
                )

            # Block-diagonal stationary first (gates the first matmul).
            em2bd_f = singles.tile([128, 128], f32)
            nc.vector.memset(em2bd_f, 0.0)
            nc.vector.tensor_scalar_mul(
                em2bd_f[0:64, 0:64], ewc2[0:64, 0:DIM], -2.0
            )
            nc.vector.tensor_scalar_mul(
                em2bd_f[64:128, 64:128], ewc2[64:128, 0:DIM], -2.0
            )
            em2bd = singles.tile([128, 128], bf16)
            nc.vector.tensor_copy(out=em2bd, in_=em2bd_f)

            # bias[m] = ||e_{m%64}||^2 (+1 for the normalized bank rows),
            # f32 via DVE square + free-dim reduce.
            sq_ewt = singles.tile([128, DIM], f32)
            nc.vector.tensor_mul(sq_ewt, ewc2[:, 128:192], ewc2[:, 128:192])
            bias = singles.tile([128, 1], f32)
            nc.vector.tensor_reduce(
                out=bias,
                in_=sq_ewt,
                axis=mybir.AxisListType.X,
                op=mybir.AluOpType.add,
            )
            if fused_norm:
                bias1 = singles.tile([128, 1], f32)
                nc.vector.tensor_scalar_add(bias1, bias, 1.0)
                bias = bias1

            if not fused_norm:
                onesbd_f = singles.tile([128, 128], f32)
                nc.vector.memset(onesbd_f, 0.0)
                nc.vector.memset(onesbd_f[0:64, 0:64], 1.0)
                nc.vector.memset(onesbd_f[64:128, 64:128], 1.0)
                onesbd = singles.tile([128, 128], bf16)
                nc.vector.tensor_copy(out=onesbd, in_=onesbd_f)

            # --- main pipeline --------------------------------------------
            for oi, ci in enumerate(ORDER):
                w = CHUNKS[ci]
                bt_c = bts[ci]
                ps = psum.tile([128, w], f32, tag="ps")
                for j in range(w // NBLK):
                    sl = slice(j * NBLK, (j + 1) * NBLK)
                    nc.tensor.matmul(
                        ps[:, sl], lhsT=em2bd, rhs=bt_c[:, sl],
                        start=True, stop=fused_norm,
                    )
                if not fused_norm:
                    sq_c = sq_pool.tile([128, w], bf16, tag="sq")
                    nc.vector.tensor_mul(sq_c, bt_c, bt_c)
                    for j in range(w // NBLK):
                        sl = slice(j * NBLK, (j + 1) * NBLK)
                        nc.tensor.matmul(
                            ps[:, sl], lhsT=onesbd, rhs=sq_c[:, sl],
                            start=False, stop=True,
                        )
                cs = slice(int(offs[ci]), int(offs[ci + 1]))
                out_c = out_pool.tile([128, w], bf16, tag="out")
                nc.scalar.activation(
                    out=out_c, in_=ps, func=SQRT, bias=bias, scale=1.0
                )
                if oi == len(ORDER) - 1:
                    nc.scalar.dma_start(out=o[:, cs], in_=out_c)
                elif oi % 2 == 0:
                    nc.gpsimd.dma_start(out=o[:, cs], in_=out_c)
                else:
                    nc.sync.dma_start(out=o[:, cs], in_=out_c)

    nc.compile()
    return nc


def _get_nc(fused_norm):
    key = ("nc", fused_norm)
    if key not in _cache:
        _cache[key] = _build(fused_norm)
    return _cache[key]


def _prep_inputs(emb_batch, bank):
    """Host-side re-layout only (shard, transpose, stack, bf16 cast)."""
    import ml_dtypes

    bf16 = ml_dtypes.bfloat16
    emb_batch = np.asarray(emb_batch, dtype=np.float32)
    bank = np.asarray(bank, dtype=np.float32)
    # [128, 192]: cols 0-127 tiled embT (both partition-halves), cols
    # 128-191 query-major emb for the on-device ||e||^2 reduce.
    ewc_host = np.ascontiguousarray(
        np.concatenate(
            [np.tile(emb_batch.T, (2, 2)), np.tile(emb_batch, (2, 1))], axis=1
        )
    )
    bankT = bank.T  # [64, BANK] view
    in_maps = []
    for c in range(N_CORES):
        sh = bankT[:, c * SHARD : (c + 1) * SHARD]
        btc = np.ascontiguousarray(
            np.concatenate([sh[:, :HALF], sh[:, HALF:]], axis=0)
        ).astype(bf16)
        in_maps.append({"bt": btc, "ewc": ewc_host})
    return in_maps


def kernel(emb_batch, bank):
    global last_run
    from concourse.bass_utils import run_bass_kernel_spmd

    bank = np.asarray(bank, dtype=np.float32)
    # Read-only validation: MemoryBank rows are L2-normalized. Use the
    # fused-constant kernel when that holds, the general one otherwise.
    norms = np.einsum("bd,bd->b", bank, bank)
    fused_norm = bool(np.abs(norms - 1.0).max() < 1e-3)

    nc = _get_nc(fused_norm)
    in_maps = _prep_inputs(emb_batch, bank)
    res = run_bass_kernel_spmd(nc, in_maps, core_ids=list(range(N_CORES)))
    last_run = res
    out = np.empty((BATCH, BANK), dtype=np.float32)
    for c in range(N_CORES):
        oc = res.results[c]["o"]  # [128, HALF] bf16: rows (h*64 + n)
        oc = np.asarray(oc).astype(np.float32)
        out[:, c * SHARD : c * SHARD + HALF] = oc[0:64]
        out[:, c * SHARD + HALF : (c + 1) * SHARD] = oc[64:128]
    return out


# revision 14
# speedup vs baseline: 1.0067x; 1.0067x over previous
"""MemoryBank.get_all_distances Trainium2 kernel.

emb_batch [64, 64] f32, bank [131072, 64] f32 -> distances [64, 131072] f32
  distances[n, b] = || bank[b] - emb[n] ||_2

Strategy: shard bank rows across 8 cores (16384 rows each). On the host we
only re-layout (transpose + stack + bf16 cast) the shard; all arithmetic
runs on device:

  dist^2[n, b] = (||e_n||^2 + ||b_b||^2) - 2 e_n . b_b

MemoryBank rows are L2-normalized (see MemoryBank._create), so ||b_b||^2
== 1 and the first term is the per-query constant ||e_n||^2 + 1, computed
on device (DVE square + reduce + add). kernel() verifies the normalization
on the host (read-only check) and falls back to a variant that computes
||b_b||^2 on device (DVE squares + ones-matmul) if it does not hold.

Per core the shard is fed as bt [128, 8192] bf16: partitions 0-63 hold dim
d of bank columns 0..8191 of the shard, partitions 64-127 of columns
8192..16383. Each 512-column block is one K=128/M=128 bf16 matmul with the
block-diagonal stationary [[-2*embT,0],[0,-2*embT]]; the ACT engine
finishes with sqrt(psum + bias) -> bf16 and the host widens to f32.

Scheduling notes (measured from perfetto traces):
 - the NRT preamble runs to ~6.5us and a DMA doorbell costs ~0.65us of the
   issuing engine, with ~1.5us to first byte and ~2.1us from last byte to
   the completion semaphore. Both HWDGE rings sustain ~240 GB/s.
 - the ACT-table loads occupy the scalar ring until ~8.5us, so all early
   traffic rides the sync ring; the scalar ring gets late chunks only.
 - embeddings travel as ONE [128, 192] tensor (tiled embT + query-major
   copy): separate small DMAs with <512B lines crawl and serialize the
   ring.
 - chunk widths are graded (512 first / last) so the serial ACT-sqrt chain
   starts as early and drains as late as possible; chunks are processed in
   measured data-availability order, not offset order.
 - output tiles use bufs=6: an output DMA's HBM write receipt (~2us) holds
   its tile hostage well past the transfer.
 - the PE boots HAM-throttled at half clock; 3 dummy matmuls warm it up.
"""

import numpy as np

BANK = 131072
DIM = 64
BATCH = 64
N_CORES = 8
SHARD = BANK // N_CORES  # 16384 bank rows per core
HALF = SHARD // 2  # 8192 columns per partition-half
NBLK = 512  # psum bank / matmul block
N_WARM = 3  # PE warm-up dummy matmuls

# chunk widths (columns per partition-half); sum == HALF
CHUNKS = [512, 512, 1024, 1024, 1024, 1024, 1024, 1024, 512, 512]
# DMA groups as (engine, [chunk indices]): each DMA costs ~1.7us of its
# ring serially plus ~1.7us to its completion semaphore, so the fixed
# costs are spread across all three queues and run in parallel.
GROUPS = [
    ("sync", [0]),
    ("scalar", [1, 2, 3]),
    ("scalar", [4, 5]),
    ("gpsimd", [6, 7, 8, 9]),
]
# processing order (availability is balanced for natural order)
ORDER = [0, 1, 2, 3, 4, 5, 6, 7, 8, 9]

_cache = {}

# test.py reads this after calling kernel() to get profiling info.
last_run = None


def _build(fused_norm=True, half=HALF):
    import concourse.mybir as mybir
    import concourse.tile as tile
    from concourse import bacc

    f32 = mybir.dt.float32
    bf16 = mybir.dt.bfloat16
    SQRT = mybir.ActivationFunctionType.Sqrt

    offs = np.concatenate([[0], np.cumsum(CHUNKS)])
    assert offs[-1] == half

    nc = bacc.Bacc(
        "TRN2", target_bir_lowering=False, debug=False, num_devices=N_CORES
    )
    bt = nc.dram_tensor("bt", [128, half], bf16, kind="ExternalInput").ap()
    # ewc: cols 0-127 = tiled embT (stationary), 128-191 = query-major emb.
    ewc = nc.dram_tensor("ewc", [128, 192], f32, kind="ExternalInput").ap()
    o = nc.dram_tensor("o", [128, half], bf16, kind="ExternalOutput").ap()

    with tile.TileContext(nc) as tc:
        with (
            tc.tile_pool(name="singles", bufs=1) as singles,
            tc.tile_pool(name="bt_pool", bufs=1) as bt_pool,
            tc.tile_pool(name="sq_pool", bufs=3) as sq_pool,
            tc.tile_pool(name="out_pool", bufs=6) as out_pool,
            tc.tile_pool(name="psum", bufs=4, space="PSUM") as psum,
        ):
            # --- input streams, all doorbells rung up front ---------------
            ewc2 = singles.tile([128, 192], f32)
            nc.sync.dma_start(out=ewc2, in_=ewc)

            # Preload the Sqrt ACT table with the production signature.
            zt = singles.tile([128, 1], f32)
            nc.vector.memset(zt, 0.0)
            warm = singles.tile([128, 1], f32)
            nc.scalar.activation(out=warm, in_=zt, func=SQRT, bias=zt, scale=1.0)

            bts = {}
            for gi, (eng_name, cis) in enumerate(GROUPS):
                lo, hi = int(offs[cis[0]]), int(offs[cis[-1] + 1])
                gt = bt_pool.tile(
                    [128, hi - lo], bf16, name=f"btg{gi}", tag=f"btg{gi}"
                )
                for ci in cis:
                    bts[ci] = gt[:, int(offs[ci]) - lo : int(offs[ci + 1]) - lo]
                eng = {"sync": nc.sync, "scalar": nc.scalar,
                       "gpsimd": nc.gpsimd}[eng_name]
                eng.dma_start(out=gt, in_=bt[:, lo:hi])

            # --- PE warm-up against the HAM clock gate --------------------
            dummy_w = singles.tile([128, 128], bf16)
            nc.gpsimd.memset(dummy_w, 0.0)
            dummy_r = singles.tile([128, 512], bf16)
            nc.gpsimd.memset(dummy_r, 0.0)
            ps_warm = psum.tile([128, 1024], f32, tag="ps")
            for _ in range(N_WARM):
                nc.tensor.matmul(
                    ps_warm[:, 0:512], lhsT=dummy_w, rhs=dummy_r,
                    start=True, stop=True,
                )

            # Block-diagonal stationary first (gates the first matmul).
            em2bd_f = singles.tile([128, 128], f32)
            nc.vector.memset(em2bd_f, 0.0)
            nc.vector.tensor_scalar_mul(
                em2bd_f[0:64, 0:64], ewc2[0:64, 0:DIM], -2.0
            )
            nc.vector.tensor_scalar_mul(
                em2bd_f[64:128, 64:128], ewc2[64:128, 0:DIM], -2.0
            )
            em2bd = singles.tile([128, 128], bf16)
            nc.vector.tensor_copy(out=em2bd, in_=em2bd_f)

            # bias[m] = ||e_{m%64}||^2 (+1 for the normalized bank rows),
            # f32 via DVE square + free-dim reduce.
            sq_ewt = singles.tile([128, DIM], f32)
            nc.vector.tensor_mul(sq_ewt, ewc2[:, 128:192], ewc2[:, 128:192])
            bias = singles.tile([128, 1], f32)
            nc.vector.tensor_reduce(
                out=bias,
                in_=sq_ewt,
                axis=mybir.AxisListType.X,
                op=mybir.AluOpType.add,
            )
            if fused_norm:
                bias1 = singles.tile([128, 1], f32)
                nc.vector.tensor_scalar_add(bias1, bias, 1.0)
                bias = bias1

            if not fused_norm:
                onesbd_f = singles.tile([128, 128], f32)
                nc.vector.memset(onesbd_f, 0.0)
                nc.vector.memset(onesbd_f[0:64, 0:64], 1.0)
                nc.vector.memset(onesbd_f[64:128, 64:128], 1.0)
                onesbd = singles.tile([128, 128], bf16)
                nc.vector.tensor_copy(out=onesbd, in_=onesbd_f)

            # --- main pipeline --------------------------------------------
            for oi, ci in enumerate(ORDER):
                w = CHUNKS[ci]
                bt_c = bts[ci]
                ps = psum.tile([128, w], f32, tag="ps")
                for j in range(w // NBLK):
                    sl = slice(j * NBLK, (j + 1) * NBLK)
                    nc.tensor.matmul(
                        ps[:, sl], lhsT=em2bd, rhs=bt_c[:, sl],
                        start=True, stop=fused_norm,
                    )
                if not fused_norm:
                    sq_c = sq_pool.tile([128, w], bf16, tag="sq")
                    nc.vector.tensor_mul(sq_c, bt_c, bt_c)
                    for j in range(w // NBLK):
                        sl = slice(j * NBLK, (j + 1) * NBLK)
                        nc.tensor.matmul(
                            ps[:, sl], lhsT=onesbd, rhs=sq_c[:, sl],
                            start=False, stop=True,
                        )
                cs = slice(int(offs[ci]), int(offs[ci + 1]))
                out_c = out_pool.tile([128, w], bf16, tag="out")
                nc.scalar.activation(
                    out=out_c, in_=ps, func=SQRT, bias=bias, scale=1.0
                )
                if oi == len(ORDER) - 1:
                    nc.scalar.dma_start(out=o[:, cs], in_=out_c)
                elif oi % 2 == 0:
                    nc.gpsimd.dma_start(out=o[:, cs], in_=out_c)
                else:
                    nc.sync.dma_start(out=o[:, cs], in_=out_c)

    nc.compile()
    return nc


def _get_nc(fused_norm):
    key = ("nc", fused_norm)
    if key not in _cache:
        _cache[key] = _build(fused_norm)
    return _cache[key]


def _prep_inputs(emb_batch, bank):
    """Host-side re-layout only (shard, transpose, stack, bf16 cast)."""
    import ml_dtypes

    bf16 = ml_dtypes.bfloat16
    emb_batch = np.asarray(emb_batch, dtype=np.float32)
    bank = np.asarray(bank, dtype=np.float32)
    # [128, 192]: cols 0-127 tiled embT (both partition-halves), cols
    # 128-191 query-major emb for the on-device ||e||^2 reduce.
    ewc_host = np.ascontiguousarray(
        np.concatenate(
            [np.tile(emb_batch.T, (2, 2)), np.tile(emb_batch, (2, 1))], axis=1
        )
    )
    bankT = bank.T  # [64, BANK] view
    in_maps = []
    for c in range(N_CORES):
        sh = bankT[:, c * SHARD : (c + 1) * SHARD]
        btc = np.ascontiguousarray(
            np.concatenate([sh[:, :HALF], sh[:, HALF:]], axis=0)
        ).astype(bf16)
        in_maps.append({"bt": btc, "ewc": ewc_host})
    return in_maps


def kernel(emb_batch, bank):
    global last_run
    from concourse.bass_utils import run_bass_kernel_spmd

    bank = np.asarray(bank, dtype=np.float32)
    # Read-only validation: MemoryBank rows are L2-normalized. Use the
    # fused-constant kernel when that holds, the general one otherwise.
    norms = np.einsum("bd,bd->b", bank, bank)
    fused_norm = bool(np.abs(norms - 1.0).max() < 1e-3)

    nc = _get_nc(fused_norm)
    in_maps = _prep_inputs(emb_batch, bank)
    res = run_bass_kernel_spmd(nc, in_maps, core_ids=list(range(N_CORES)))
    last_run = res
    out = np.empty((BATCH, BANK), dtype=np.float32)
    for c in range(N_CORES):
        oc = res.results[c]["o"]  # [128, HALF] bf16: rows (h*64 + n)
        oc = np.asarray(oc).astype(np.float32)
        out[:, c * SHARD : c * SHARD + HALF] = oc[0:64]
        out[:, c * SHARD + HALF : (c + 1) * SHARD] = oc[64:128]
    return out


# revision 15
# speedup vs baseline: 1.0388x; 1.0319x over previous
"""MemoryBank.get_all_distances Trainium2 kernel.

emb_batch [64, 64] f32, bank [131072, 64] f32 -> distances [64, 131072] f32
  distances[n, b] = || bank[b] - emb[n] ||_2

Strategy: shard bank rows across 8 cores (16384 rows each). On the host we
only re-layout (transpose + stack + bf16 cast) the shard; all arithmetic
runs on device:

  dist^2[n, b] = (||e_n||^2 + ||b_b||^2) - 2 e_n . b_b

MemoryBank rows are L2-normalized (see MemoryBank._create), so ||b_b||^2
== 1 and the first term is the per-query constant ||e_n||^2 + 1, computed
on device (DVE square + reduce + add). kernel() verifies the normalization
on the host (read-only check) and falls back to a variant that computes
||b_b||^2 on device (DVE squares + ones-matmul) if it does not hold.

Per core the shard is fed as bt [128, 8192] bf16: partitions 0-63 hold dim
d of bank columns 0..8191 of the shard, partitions 64-127 of columns
8192..16383. Each 512-column block is one K=128/M=128 bf16 matmul with the
block-diagonal stationary [[-2*embT,0],[0,-2*embT]]; the ACT engine
finishes with sqrt(psum + bias) -> bf16 and the host widens to f32.

Scheduling notes (measured from perfetto traces):
 - the NRT preamble runs to ~6.5us and a DMA doorbell costs ~0.65us of the
   issuing engine, with ~1.5us to first byte and ~2.1us from last byte to
   the completion semaphore. Both HWDGE rings sustain ~240 GB/s.
 - the ACT-table loads occupy the scalar ring until ~8.5us, so all early
   traffic rides the sync ring; the scalar ring gets late chunks only.
 - embeddings travel as ONE [128, 192] tensor (tiled embT + query-major
   copy): separate small DMAs with <512B lines crawl and serialize the
   ring.
 - chunk widths are graded (512 first / last) so the serial ACT-sqrt chain
   starts as early and drains as late as possible; chunks are processed in
   measured data-availability order, not offset order.
 - output tiles use bufs=6: an output DMA's HBM write receipt (~2us) holds
   its tile hostage well past the transfer.
 - the PE boots HAM-throttled at half clock; 3 dummy matmuls warm it up.
"""

import numpy as np

BANK = 131072
DIM = 64
BATCH = 64
N_CORES = 8
SHARD = BANK // N_CORES  # 16384 bank rows per core
HALF = SHARD // 2  # 8192 columns per partition-half
NBLK = 512  # psum bank / matmul block
N_WARM = 3  # PE warm-up dummy matmuls

# chunk widths (columns per partition-half); sum == HALF
CHUNKS = [512, 512, 1024, 1024, 1024, 1024, 1024, 1024, 512, 512]
# DMA groups as (engine, [chunk indices]): each DMA costs ~1.7us of its
# ring serially plus ~1.7us to its completion semaphore, so the fixed
# costs are spread across all three queues and run in parallel.
GROUPS = [
    ("sync", [0]),
    ("sync", [1, 2]),
    ("sync", [3, 4]),
    ("scalar", [5, 6]),
    ("scalar", [7, 8, 9]),
]
# processing order: by measured availability (group sem times)
ORDER = [0, 1, 2, 5, 6, 3, 4, 7, 8, 9]

_cache = {}

# test.py reads this after calling kernel() to get profiling info.
last_run = None


def _build(fused_norm=True, half=HALF):
    import concourse.mybir as mybir
    import concourse.tile as tile
    from concourse import bacc

    f32 = mybir.dt.float32
    bf16 = mybir.dt.bfloat16
    SQRT = mybir.ActivationFunctionType.Sqrt

    offs = np.concatenate([[0], np.cumsum(CHUNKS)])
    assert offs[-1] == half

    nc = bacc.Bacc(
        "TRN2", target_bir_lowering=False, debug=False, num_devices=N_CORES
    )
    bt = nc.dram_tensor("bt", [128, half], bf16, kind="ExternalInput").ap()
    # ewc: cols 0-127 = tiled embT (stationary), 128-191 = query-major emb.
    ewc = nc.dram_tensor("ewc", [128, 192], f32, kind="ExternalInput").ap()
    o = nc.dram_tensor("o", [128, half], bf16, kind="ExternalOutput").ap()

    with tile.TileContext(nc) as tc:
        with (
            tc.tile_pool(name="singles", bufs=1) as singles,
            tc.tile_pool(name="bt_pool", bufs=1) as bt_pool,
            tc.tile_pool(name="sq_pool", bufs=3) as sq_pool,
            tc.tile_pool(name="out_pool", bufs=6) as out_pool,
            tc.tile_pool(name="psum", bufs=4, space="PSUM") as psum,
        ):
            # --- input streams, all doorbells rung up front ---------------
            ewc2 = singles.tile([128, 192], f32)
            nc.sync.dma_start(out=ewc2, in_=ewc)

            # Preload the Sqrt ACT table with the production signature.
            zt = singles.tile([128, 1], f32)
            nc.vector.memset(zt, 0.0)
            warm = singles.tile([128, 1], f32)
            nc.scalar.activation(out=warm, in_=zt, func=SQRT, bias=zt, scale=1.0)

            bts = {}
            for gi, (eng_name, cis) in enumerate(GROUPS):
                lo, hi = int(offs[cis[0]]), int(offs[cis[-1] + 1])
                gt = bt_pool.tile(
                    [128, hi - lo], bf16, name=f"btg{gi}", tag=f"btg{gi}"
                )
                for ci in cis:
                    bts[ci] = gt[:, int(offs[ci]) - lo : int(offs[ci + 1]) - lo]
                eng = {"sync": nc.sync, "scalar": nc.scalar,
                       "gpsimd": nc.gpsimd}[eng_name]
                eng.dma_start(out=gt, in_=bt[:, lo:hi])

            # --- PE warm-up against the HAM clock gate --------------------
            dummy_w = singles.tile([128, 128], bf16)
            nc.gpsimd.memset(dummy_w, 0.0)
            dummy_r = singles.tile([128, 512], bf16)
            nc.gpsimd.memset(dummy_r, 0.0)
            ps_warm = psum.tile([128, 1024], f32, tag="ps")
            for _ in range(N_WARM):
                nc.tensor.matmul(
                    ps_warm[:, 0:512], lhsT=dummy_w, rhs=dummy_r,
                    start=True, stop=True,
                )

            # Block-diagonal stationary first (gates the first matmul).
            em2bd_f = singles.tile([128, 128], f32)
            nc.vector.memset(em2bd_f, 0.0)
            nc.vector.tensor_scalar_mul(
                em2bd_f[0:64, 0:64], ewc2[0:64, 0:DIM], -2.0
            )
            nc.vector.tensor_scalar_mul(
                em2bd_f[64:128, 64:128], ewc2[64:128, 0:DIM], -2.0
            )
            em2bd = singles.tile([128, 128], bf16)
            nc.vector.tensor_copy(out=em2bd, in_=em2bd_f)

            # bias[m] = ||e_{m%64}||^2 (+1 for the normalized bank rows),
            # f32 via DVE square + free-dim reduce.
            sq_ewt = singles.tile([128, DIM], f32)
            nc.vector.tensor_mul(sq_ewt, ewc2[:, 128:192], ewc2[:, 128:192])
            bias = singles.tile([128, 1], f32)
            nc.vector.tensor_reduce(
                out=bias,
                in_=sq_ewt,
                axis=mybir.AxisListType.X,
                op=mybir.AluOpType.add,
            )
            if fused_norm:
                bias1 = singles.tile([128, 1], f32)
                nc.vector.tensor_scalar_add(bias1, bias, 1.0)
                bias = bias1

            if not fused_norm:
                onesbd_f = singles.tile([128, 128], f32)
                nc.vector.memset(onesbd_f, 0.0)
                nc.vector.memset(onesbd_f[0:64, 0:64], 1.0)
                nc.vector.memset(onesbd_f[64:128, 64:128], 1.0)
                onesbd = singles.tile([128, 128], bf16)
                nc.vector.tensor_copy(out=onesbd, in_=onesbd_f)

            # --- main pipeline --------------------------------------------
            for oi, ci in enumerate(ORDER):
                w = CHUNKS[ci]
                bt_c = bts[ci]
                ps = psum.tile([128, w], f32, tag="ps")
                for j in range(w // NBLK):
                    sl = slice(j * NBLK, (j + 1) * NBLK)
                    nc.tensor.matmul(
                        ps[:, sl], lhsT=em2bd, rhs=bt_c[:, sl],
                        start=True, stop=fused_norm,
                    )
                if not fused_norm:
                    sq_c = sq_pool.tile([128, w], bf16, tag="sq")
                    nc.vector.tensor_mul(sq_c, bt_c, bt_c)
                    for j in range(w // NBLK):
                        sl = slice(j * NBLK, (j + 1) * NBLK)
                        nc.tensor.matmul(
                            ps[:, sl], lhsT=onesbd, rhs=sq_c[:, sl],
                            start=False, stop=True,
                        )
                cs = slice(int(offs[ci]), int(offs[ci + 1]))
                out_c = out_pool.tile([128, w], bf16, tag="out")
                nc.scalar.activation(
                    out=out_c, in_=ps, func=SQRT, bias=bias, scale=1.0
                )
                if oi == len(ORDER) - 1:
                    nc.scalar.dma_start(out=o[:, cs], in_=out_c)
                elif oi % 2 == 0:
                    nc.gpsimd.dma_start(out=o[:, cs], in_=out_c)
                else:
                    nc.sync.dma_start(out=o[:, cs], in_=out_c)

    nc.compile()
    return nc


def _get_nc(fused_norm):
    key = ("nc", fused_norm)
    if key not in _cache:
        _cache[key] = _build(fused_norm)
    return _cache[key]


def _prep_inputs(emb_batch, bank):
    """Host-side re-layout only (shard, transpose, stack, bf16 cast)."""
    import ml_dtypes

    bf16 = ml_dtypes.bfloat16
    emb_batch = np.asarray(emb_batch, dtype=np.float32)
    bank = np.asarray(bank, dtype=np.float32)
    # [128, 192]: cols 0-127 tiled embT (both partition-halves), cols
    # 128-191 query-major emb for the on-device ||e||^2 reduce.
    ewc_host = np.ascontiguousarray(
        np.concatenate(
            [np.tile(emb_batch.T, (2, 2)), np.tile(emb_batch, (2, 1))], axis=1
        )
    )
    bankT = bank.T  # [64, BANK] view
    in_maps = []
    for c in range(N_CORES):
        sh = bankT[:, c * SHARD : (c + 1) * SHARD]
        btc = np.ascontiguousarray(
            np.concatenate([sh[:, :HALF], sh[:, HALF:]], axis=0)
        ).astype(bf16)
        in_maps.append({"bt": btc, "ewc": ewc_host})
    return in_maps


def kernel(emb_batch, bank):
    global last_run
    from concourse.bass_utils import run_bass_kernel_spmd

    bank = np.asarray(bank, dtype=np.float32)
    # Read-only validation: MemoryBank rows are L2-normalized. Use the
    # fused-constant kernel when that holds, the general one otherwise.
    norms = np.einsum("bd,bd->b", bank, bank)
    fused_norm = bool(np.abs(norms - 1.0).max() < 1e-3)

    nc = _get_nc(fused_norm)
    in_maps = _prep_inputs(emb_batch, bank)
    res = run_bass_kernel_spmd(nc, in_maps, core_ids=list(range(N_CORES)))
    last_run = res
    out = np.empty((BATCH, BANK), dtype=np.float32)
    for c in range(N_CORES):
        oc = res.results[c]["o"]  # [128, HALF] bf16: rows (h*64 + n)
        oc = np.asarray(oc).astype(np.float32)
        out[:, c * SHARD : c * SHARD + HALF] = oc[0:64]
        out[:, c * SHARD + HALF : (c + 1) * SHARD] = oc[64:128]
    return out
